# revision 32
# baseline (speedup 1.0000x reference)
"""Butterfly block-sparse linear kernel for Trainium2 (8 NeuronCores, SPMD).

Computes: y = blockdiag_butterfly(x, factorL, factorR) + bias
  x:(4,2048,4096) f32, factorL/factorR:(8,512,512) f32, bias:(4096,) f32

Math (reference):
  out1[b,k,q] = sum_p x[b, 512k+p] * factorL[k,q,p]      (8 blocks of 512x512)
  z[b,l,r]    = out1_flat[b, 8r+l]                        (butterfly permute)
  out2[b,l,s] = sum_r z[b,l,r] * factorR[l,s,r]
  y[b, 8s+l]  = out2[b,l,s] + bias[8s+l]

Strategy: data-parallel over the 8192 tokens (1024 tokens/core), factors
replicated. All activations are kept feature-major on chip (features on
SBUF partitions, tokens on the free axis) so both block matmuls contract
over the partition dim. The butterfly permute becomes:
  - a host-side reordering of factorL's output channels q -> q' = 64*(q%8)+q//8
    (groups stage-1 channels by their destination stage-2 block l), and
  - an on-chip gather: each stage-1 PSUM tile (128 q' x T) splits into two
    64-partition halves (block l=2qc and l=2qc+1), which DMA (SBUF->SBUF,
    partition-remapped) into the stage-2 input tiles z[l][c].
Matmuls run as float32r (full PE rate for moving dim >= 256, ~1e-4 rel err).
Stage-2 output is evicted by ScalarE with the per-partition bias fused, then
DMA'd to HBM with row stride 8 so the final feature order j = 8s+l is already
correct; the host only transposes token-major at the end.
"""

import os
import numpy as np
from contextlib import ExitStack

NCORES = 8
TOK = 8192
TPC = TOK // NCORES          # tokens per core
TBATCH = 512                 # tokens per on-chip batch
NB = TPC // TBATCH

_CACHE = {}
LAST_RESULT = None


def _build_program():
    import concourse.bacc as bacc
    import concourse.tile as tile
    import concourse.mybir as mybir

    F32 = mybir.dt.float32
    BF16 = mybir.dt.bfloat16

    nc = bacc.Bacc("TRN2", target_bir_lowering=False, debug=False)
    x = nc.dram_tensor("x", [4096, TPC], BF16, kind="ExternalInput").ap()
    w1 = nc.dram_tensor("w1", [128, 16384], BF16, kind="ExternalInput").ap()
    w2 = nc.dram_tensor("w2", [128, 16384], BF16, kind="ExternalInput").ap()
    bias = nc.dram_tensor("bias", [128, 32], F32, kind="ExternalInput").ap()
    out = nc.dram_tensor("out", [4096, TPC], BF16, kind="ExternalOutput").ap()
    # out rows j = 1024*sc + 8*ss + l  ->  view as [sc, l, ss, t]
    out_r = out.rearrange("(a p l) t -> a l p t", p=128, l=8)

    T = TBATCH
    # x viewed per k-group: [k, pc, pp, t]
    x_r = x.rearrange("(k pc pp) t -> k pp pc t", pc=4, pp=128)

    with tile.TileContext(nc) as tc, ExitStack() as ctx:
        wpool = ctx.enter_context(tc.tile_pool(name="w", bufs=1))
        w1pool = ctx.enter_context(tc.tile_pool(name="w1p", bufs=1))
        w2pool = ctx.enter_context(tc.tile_pool(name="w2p", bufs=1))
        xpool = ctx.enter_context(tc.tile_pool(name="x", bufs=4))
        spool = ctx.enter_context(tc.tile_pool(name="stg", bufs=6))
        zpool = ctx.enter_context(tc.tile_pool(name="z", bufs=2))
        opool = ctx.enter_context(tc.tile_pool(name="o", bufs=2))
        ps1 = ctx.enter_context(tc.tile_pool(name="ps1", bufs=3, space="PSUM"))
        ps2 = ctx.enter_context(tc.tile_pool(name="ps2", bufs=5, space="PSUM"))

        bt = wpool.tile([128, 32], F32, tag="bias")
        nc.scalar.dma_start(bt[:], bias[:])
        # w1/w2 stay resident all kernel; loads are paced into HBM-idle windows
        w2ts = [
            w2pool.tile([128, 2048], BF16, name=f"w2_{l}", tag=f"w2_{l}")
            for l in range(8)
        ]
        w1ts = [
            w1pool.tile([128, 2048], BF16, name=f"w1_{k}", tag=f"w1_{k}")
            for k in range(8)
        ]

        # All loads are emitted up front on dedicated queues; tile-pool WAR
        # semaphores provide the prefetch pacing automatically.
        #   gpsimd: x(b0), w2, x(b1)   (pure loads, in need-order)
        #   sync:   w1, then gathers + stores (ready-ordered, never block)
        #   scalar: bias only (ACT engine stays free for evictions)
        loads = {}

        def emit_load(b, k):
            t0 = b * T
            if b == 0 and k == 0:
                # first k: two half tiles so matmul 0 waits on less data
                xta = wpool.tile([128, 2 * T], BF16, name="x00a", tag="x00a")
                xtb = wpool.tile([128, 2 * T], BF16, name="x00b", tag="x00b")
                nc.gpsimd.dma_start(xta[:], x_r[k, :, 0:2, t0 : t0 + T])
                nc.gpsimd.dma_start(xtb[:], x_r[k, :, 2:4, t0 : t0 + T])
                loads[(b, k)] = (
                    xta[:, 0:T], xta[:, T : 2 * T],
                    xtb[:, 0:T], xtb[:, T : 2 * T],
                )
            else:
                xt = xpool.tile([128, 4 * T], BF16, tag="xt")
                nc.gpsimd.dma_start(
                    xt[:].rearrange("p (c t) -> p c t", c=4),
                    x_r[k, :, :, t0 : t0 + T],
                )
                loads[(b, k)] = tuple(
                    xt[:, pc * T : (pc + 1) * T] for pc in range(4)
                )

        def s1_compute(b, k):
            xpcs = loads.pop((b, k))
            # Each qc PSUM tile splits into an aligned half (same partition
            # range as its z destination -> engine-copied directly, no DMA)
            # and a crossed half (staged, then one partition-remap DMA per k).
            # Aligned l-parity == k-parity. Even qc on DVE, odd qc on ACT so
            # the two engines never share a PSUM bank.
            c, h = k // 2, 64 * (k % 2)
            hx = 64 - h
            zv = zts[c].rearrange("p (l t) -> p l t", l=8)
            stg = spool.tile([128, 4 * T], BF16, tag="stg")
            for qc in range(4):
                p1 = ps1.tile([128, T], F32, tag="p1")
                for pc in range(4):
                    col = pc * 512 + qc * 128
                    nc.tensor.matmul(
                        p1[:],
                        w1ts[k][:, col : col + 128],
                        xpcs[pc],
                        start=(pc == 0),
                        stop=(pc == 3),
                    )
                l_a = 2 * qc + (k % 2)
                za = zv[h : h + 64, l_a, :]
                # crossed half first so the gather DMA can start earlier
                if qc % 2 == 0:
                    nc.vector.tensor_copy(
                        stg[hx : hx + 64, qc * T : (qc + 1) * T],
                        p1[hx : hx + 64, :],
                    )
                    nc.vector.tensor_copy(za, p1[h : h + 64, :])
                else:
                    nc.scalar.activation(
                        stg[hx : hx + 64, qc * T : (qc + 1) * T],
                        p1[hx : hx + 64, :],
                        mybir.ActivationFunctionType.Identity,
                    )
                    nc.scalar.activation(
                        za, p1[h : h + 64, :],
                        mybir.ActivationFunctionType.Identity,
                    )
                if k >= 6:
                    # last k-pair: per-qc gather pieces minimize the
                    # stage-1 -> stage-2 barrier latency
                    nc.sync.dma_start(
                        zv[h : h + 64, 2 * qc + (1 - k % 2), :],
                        stg[hx : hx + 64, qc * T : (qc + 1) * T],
                    )
            if k < 6:
                nc.sync.dma_start(
                    zv[h : h + 64, (1 - k % 2) : 8 : 2, :],
                    stg[hx : hx + 64, :].rearrange("p (q t) -> p q t", q=4),
                )

        def s2_compute(b, l):
            t0 = b * T
            split = b == NB - 1 and l == L_ORDER[-1]
            ot = opool.tile([128, 4 * T], BF16, tag="ot")
            for sc in range(4):
                p2 = ps2.tile([128, T], F32, tag="p2")
                for c in range(4):
                    col = c * 512 + sc * 128
                    nc.tensor.matmul(
                        p2[:],
                        w2ts[l][:, col : col + 128],
                        zts[c][:, l * T : (l + 1) * T],
                        start=(c == 0),
                        stop=(c == 3),
                    )
                # eviction split DVE/ACT so the PE is not gated on one
                # engine's eviction pace (DVE adds bias via tensor_scalar)
                if sc % 2 == 0:
                    nc.vector.tensor_scalar_add(
                        ot[:, sc * T : (sc + 1) * T],
                        p2[:],
                        bt[:, l * 4 + sc : l * 4 + sc + 1],
                    )
                else:
                    nc.scalar.activation(
                        ot[:, sc * T : (sc + 1) * T],
                        p2[:],
                        mybir.ActivationFunctionType.Identity,
                        bias=bt[:, l * 4 + sc : l * 4 + sc + 1],
                    )
                if split:
                    # quarter stores, alternating queues, so the final store
                    # fully overlaps the last evictions
                    qs = nc.scalar if sc % 2 == 0 else nc.sync
                    qs.dma_start(
                        out_r[sc, l, :, t0 : t0 + T],
                        ot[:, sc * T : (sc + 1) * T],
                    )
            # one store per l: rows j = 1024*sc + 8*ss + l, cols t0:t0+T
            if not split:
                nc.sync.dma_start(
                    out_r[:, l, :, t0 : t0 + T].rearrange("a p t -> p a t"),
                    ot[:].rearrange("p (a t) -> p a t", a=4),
                )

        # stage-2 visits odd l first: their z deps (gather of k=6 + aligned
        # copies of k=7) complete before the even-l gather of k=7 lands
        L_ORDER = [1, 3, 5, 7, 0, 2, 4, 6]
        # every load emitted up front, in need-order per queue; xpool WAR
        # (bufs=4) paces the x stream, w1/w2 are resident
        nc.sync.dma_start(w1ts[0][:, 0:1024], w1[:, 0:1024])
        nc.sync.dma_start(w1ts[0][:, 1024:2048], w1[:, 1024:2048])
        for k in range(1, 8):
            nc.sync.dma_start(w1ts[k][:], w1[:, k * 2048 : (k + 1) * 2048])
        for k in range(8):
            emit_load(0, k)
        for lw in L_ORDER:
            nc.gpsimd.dma_start(w2ts[lw][:], w2[:, lw * 2048 : (lw + 1) * 2048])
        for k in range(8):
            emit_load(1, k)
        for b in range(NB):
            t0 = b * T
            # z split per r-chunk c: tile c holds [l, t] slots for r-rows
            # [128c, 128c+128); written by k=2c (parts 0:64) and k=2c+1
            zts = [
                zpool.tile([128, 8 * T], BF16, name=f"z_{c}", tag=f"z_{c}")
                for c in range(4)
            ]
            for k in range(8):
                s1_compute(b, k)
            for j in range(8):
                s2_compute(b, L_ORDER[j])
    nc.compile()
    return nc


def _get_program():
    if "nc" not in _CACHE:
        _CACHE["nc"] = _build_program()
    return _CACHE["nc"]


def _ensure_ntff_hook():
    """Bridge the axon NTFF profile hook when the image's antenv lacks it."""
    import sys, types

    try:
        from antenv.axon_hooks import get_axon_ntff_profile_hook  # noqa: F401

        return
    except ImportError:
        pass
    try:
        from trn_agent_boot.trn_boot import _ntff_profile_via_ctypes

        hook = _ntff_profile_via_ctypes("/opt/axon/libaxon_pjrt.so")
        mod = types.ModuleType("antenv.axon_hooks")
        _h = {"hook": hook}
        mod.set_axon_ntff_profile_hook = lambda h: _h.__setitem__("hook", h)
        mod.get_axon_ntff_profile_hook = lambda: _h["hook"]
        sys.modules["antenv.axon_hooks"] = mod
        import antenv

        antenv.axon_hooks = mod
    except Exception:
        pass


def kernel(x, factorL, factorR, bias):
    global LAST_RESULT
    import ml_dtypes
    from concourse.bass_utils import run_bass_kernel_spmd

    BF = ml_dtypes.bfloat16
    x = np.asarray(x, dtype=np.float32)
    factorL = np.asarray(factorL, dtype=np.float32)
    factorR = np.asarray(factorR, dtype=np.float32)
    bias = np.asarray(bias, dtype=np.float32)

    # host-side marshalling (not device-timed)
    xt = np.ascontiguousarray(x.reshape(TOK, 4096).T.astype(BF))  # (4096, 8192)
    qp = np.arange(512)
    q_of_qprime = 8 * (qp % 64) + qp // 64
    w1p = factorL.transpose(0, 2, 1)[:, :, q_of_qprime]  # (8, p, q')
    w1dev = np.ascontiguousarray(
        w1p.reshape(8, 4, 128, 4, 128)
        .transpose(2, 0, 1, 3, 4)
        .reshape(128, 16384)
        .astype(BF)
    )
    w2p = factorR.transpose(0, 2, 1)  # (8, r, s)
    w2dev = np.ascontiguousarray(
        w2p.reshape(8, 4, 128, 4, 128)
        .transpose(2, 0, 1, 3, 4)
        .reshape(128, 16384)
        .astype(BF)
    )
    biasdev = np.ascontiguousarray(
        bias.reshape(4, 128, 8).transpose(1, 2, 0).reshape(128, 32)
    )

    in_maps = [
        {
            "x": np.ascontiguousarray(xt[:, c * TPC : (c + 1) * TPC]),
            "w1": w1dev,
            "w2": w2dev,
            "bias": biasdev,
        }
        for c in range(NCORES)
    ]
    nc = _get_program()
    trace = os.environ.get("BUTTERFLY_TRACE", "0") == "1"
    if trace:
        _ensure_ntff_hook()
    LAST_RESULT = run_bass_kernel_spmd(
        nc, in_maps, list(range(NCORES)), trace=trace
    )
    yt = np.concatenate(
        [LAST_RESULT.results[c]["out"] for c in range(NCORES)], axis=1
    )  # (4096, 8192) bf16
    return np.ascontiguousarray(yt.T).astype(np.float32).reshape(4, 2048, 4096)



# revision 36
# speedup vs baseline: 1.0888x; 1.0888x over previous
"""Butterfly block-sparse linear kernel for Trainium2 (8 NeuronCores, SPMD).

Computes: y = blockdiag_butterfly(x, factorL, factorR) + bias
  x:(4,2048,4096) f32, factorL/factorR:(8,512,512) f32, bias:(4096,) f32

Math (reference):
  out1[b,k,q] = sum_p x[b, 512k+p] * factorL[k,q,p]      (8 blocks of 512x512)
  z[b,l,r]    = out1_flat[b, 8r+l]                        (butterfly permute)
  out2[b,l,s] = sum_r z[b,l,r] * factorR[l,s,r]
  y[b, 8s+l]  = out2[b,l,s] + bias[8s+l]

Strategy: data-parallel over the 8192 tokens (1024 tokens/core), factors
replicated, everything bf16 on the wire (PSUM accumulates fp32; harness
tolerance is 2e-2, bf16 lands ~4e-3). All activations are feature-major on
chip (features on SBUF partitions, tokens on the free axis).

The butterfly permute costs no data movement at all: stage-1's output
channels are pre-permuted on the host with a k-DEPENDENT layout
    q'' = 128*(l//2) + 64*((l+k)%2) + r_local        (q = 8*r_local + l)
so each stage-2 contraction chunk (l, k) lands in a fixed 64-partition half
(64*((l+k)%2)) of stage-1 block k's output. Stage-1 PSUM tiles are evicted
with plain full-tile copies (no split, no partition remap), and stage-2
contracts with 8 K=64 matmuls per PSUM group whose row halves alternate, so
consecutive matmuls occupy disjoint PE row-groups and run concurrently
(K-split row tiling) at the same throughput as 4 K=128 matmuls, reading the
stage-1 outputs in place.

Queues: gpsimd = x + w2 loads, sync = w1 loads + stores, scalar = bias.
All loads are emitted up front; tile-pool WAR semaphores pace the x stream.
PSUM evictions alternate DVE (tensor_scalar add for bias) / ACT so the PE
is never gated on a single engine's eviction pace. Stage-2 output is
evicted with the per-partition bias fused and stored with row stride 8 so
the final feature order j = 8s+l is already correct; the host only
transposes token-major at the end.
"""

import os
import numpy as np
from contextlib import ExitStack

NCORES = 8
TOK = 8192
TPC = TOK // NCORES          # tokens per core
TBATCH = 512                 # tokens per on-chip batch
NB = TPC // TBATCH

_CACHE = {}
LAST_RESULT = None


def _build_program():
    import concourse.bacc as bacc
    import concourse.tile as tile
    import concourse.mybir as mybir

    F32 = mybir.dt.float32
    BF16 = mybir.dt.bfloat16

    nc = bacc.Bacc("TRN2", target_bir_lowering=False, debug=False)
    x = nc.dram_tensor("x", [4096, TPC], BF16, kind="ExternalInput").ap()
    w1 = nc.dram_tensor("w1", [128, 16384], BF16, kind="ExternalInput").ap()
    w2 = nc.dram_tensor("w2", [128, 16384], BF16, kind="ExternalInput").ap()
    bias = nc.dram_tensor("bias", [128, 32], F32, kind="ExternalInput").ap()
    out = nc.dram_tensor("out", [4096, TPC], BF16, kind="ExternalOutput").ap()
    # out rows j = 1024*sc + 8*ss + l  ->  view as [sc, l, ss, t]
    out_r = out.rearrange("(a p l) t -> a l p t", p=128, l=8)

    T = TBATCH
    # x viewed per k-group: [k, pp, pc, t]
    x_r = x.rearrange("(k pc pp) t -> k pp pc t", pc=4, pp=128)

    with tile.TileContext(nc) as tc, ExitStack() as ctx:
        wpool = ctx.enter_context(tc.tile_pool(name="w", bufs=1))
        w1pool = ctx.enter_context(tc.tile_pool(name="w1p", bufs=1))
        w2pool = ctx.enter_context(tc.tile_pool(name="w2p", bufs=1))
        xpool = ctx.enter_context(tc.tile_pool(name="x", bufs=4))
        zpool = ctx.enter_context(tc.tile_pool(name="z", bufs=2))
        opool = ctx.enter_context(tc.tile_pool(name="o", bufs=2))
        ps1 = ctx.enter_context(tc.tile_pool(name="ps1", bufs=3, space="PSUM"))
        ps2 = ctx.enter_context(tc.tile_pool(name="ps2", bufs=2, space="PSUM"))

        bt = wpool.tile([128, 32], F32, tag="bias")
        nc.scalar.dma_start(bt[:], bias[:])
        # w1/w2 stay resident all kernel
        w2ts = [
            w2pool.tile([128, 4096], BF16, name=f"w2_{m}", tag=f"w2_{m}")
            for m in range(4)
        ]
        w1ts = [
            w1pool.tile([128, 2048], BF16, name=f"w1_{k}", tag=f"w1_{k}")
            for k in range(8)
        ]

        loads = {}

        def emit_load(b, k):
            t0 = b * T
            if b == 0 and k == 0:
                # first k: two half tiles so matmul 0 waits on less data
                xta = wpool.tile([128, 2 * T], BF16, name="x00a", tag="x00a")
                xtb = wpool.tile([128, 2 * T], BF16, name="x00b", tag="x00b")
                nc.gpsimd.dma_start(xta[:], x_r[k, :, 0:2, t0 : t0 + T])
                nc.gpsimd.dma_start(xtb[:], x_r[k, :, 2:4, t0 : t0 + T])
                loads[(b, k)] = (
                    xta[:, 0:T], xta[:, T : 2 * T],
                    xtb[:, 0:T], xtb[:, T : 2 * T],
                )
            else:
                xt = xpool.tile([128, 4 * T], BF16, tag="xt")
                nc.gpsimd.dma_start(
                    xt[:].rearrange("p (c t) -> p c t", c=4),
                    x_r[k, :, :, t0 : t0 + T],
                )
                loads[(b, k)] = tuple(
                    xt[:, pc * T : (pc + 1) * T] for pc in range(4)
                )

        def s1_compute(b, k):
            xpcs = loads.pop((b, k))
            zk = zkts[k]
            for qc in range(4):
                p1 = ps1.tile([128, T], F32, tag="p1")
                for pc in range(4):
                    col = pc * 512 + qc * 128
                    nc.tensor.matmul(
                        p1[:],
                        w1ts[k][:, col : col + 128],
                        xpcs[pc],
                        start=(pc == 0),
                        stop=(pc == 3),
                    )
                # full-tile eviction, alternating engines (PSUM banks never
                # shared between DVE and ACT)
                if qc % 2 == 0:
                    nc.vector.tensor_copy(zk[:, qc * T : (qc + 1) * T], p1[:])
                else:
                    nc.scalar.activation(
                        zk[:, qc * T : (qc + 1) * T],
                        p1[:],
                        mybir.ActivationFunctionType.Identity,
                    )

        def s2_compute(b, m):
            # pair (l=2m, l=2m+1): their chunk-j row halves are opposite, so
            # interleaving the two contraction streams makes consecutive
            # matmuls occupy disjoint PE row-groups AND disjoint PSUM banks
            # -> documented row-tiling concurrency, K=64 pairs run at the
            # same throughput as one K=128 matmul.
            t0 = b * T
            la, lb = 2 * m, 2 * m + 1
            split = b == NB - 1 and m == 3
            ota = opool.tile([128, 4 * T], BF16, tag="ota")
            otb = opool.tile([128, 4 * T], BF16, tag="otb")
            for sc in range(4):
                p2a = ps2.tile([128, T], F32, tag="p2a")
                p2b = ps2.tile([128, T], F32, tag="p2b", bufs=2)
                for j in range(8):
                    ha = 64 * (j % 2)
                    hb = 64 - ha
                    col = j * 512 + sc * 128
                    nc.tensor.matmul(
                        p2a[:],
                        w2ts[m][ha : ha + 64, col : col + 128],
                        zkts[j][ha : ha + 64, m * T : (m + 1) * T],
                        start=(j == 0),
                        stop=(j == 7),
                    )
                    nc.tensor.matmul(
                        p2b[:],
                        w2ts[m][hb : hb + 64, col : col + 128],
                        zkts[j][hb : hb + 64, m * T : (m + 1) * T],
                        start=(j == 0),
                        stop=(j == 7),
                    )
                # evictions: one group on DVE (bias via tensor_scalar), the
                # other on ACT, so the PE is never gated on a single engine
                nc.vector.tensor_scalar_add(
                    ota[:, sc * T : (sc + 1) * T],
                    p2a[:],
                    bt[:, la * 4 + sc : la * 4 + sc + 1],
                )
                nc.scalar.activation(
                    otb[:, sc * T : (sc + 1) * T],
                    p2b[:],
                    mybir.ActivationFunctionType.Identity,
                    bias=bt[:, lb * 4 + sc : lb * 4 + sc + 1],
                )
                if split:
                    # quarter stores, alternating queues, so the final
                    # stores fully overlap the last evictions
                    nc.sync.dma_start(
                        out_r[sc, la, :, t0 : t0 + T],
                        ota[:, sc * T : (sc + 1) * T],
                    )
                    nc.scalar.dma_start(
                        out_r[sc, lb, :, t0 : t0 + T],
                        otb[:, sc * T : (sc + 1) * T],
                    )
            # one store per l: rows j = 1024*sc + 8*ss + l, cols t0:t0+T
            if not split:
                nc.sync.dma_start(
                    out_r[:, la, :, t0 : t0 + T].rearrange("a p t -> p a t"),
                    ota[:].rearrange("p (a t) -> p a t", a=4),
                )
                nc.sync.dma_start(
                    out_r[:, lb, :, t0 : t0 + T].rearrange("a p t -> p a t"),
                    otb[:].rearrange("p (a t) -> p a t", a=4),
                )

        # every load emitted up front, in need-order per queue; xpool WAR
        # (bufs=4) paces the x stream, w1/w2 are resident
        nc.sync.dma_start(w1ts[0][:, 0:1024], w1[:, 0:1024])
        nc.sync.dma_start(w1ts[0][:, 1024:2048], w1[:, 1024:2048])
        for k in range(1, 8):
            nc.sync.dma_start(w1ts[k][:], w1[:, k * 2048 : (k + 1) * 2048])
        for k in range(8):
            emit_load(0, k)
        for m in range(4):
            nc.gpsimd.dma_start(w2ts[m][:], w2[:, m * 4096 : (m + 1) * 4096])
        for k in range(8):
            emit_load(1, k)
        for b in range(NB):
            # zk[k]: stage-1 block k's output, [128 part, qc, t]; stage 2
            # reads it in place (the host permute already placed every
            # (l, k) chunk in its 64-partition half)
            zkts = [
                zpool.tile([128, 4 * T], BF16, name=f"z_{k}", tag=f"z_{k}")
                for k in range(8)
            ]
            for k in range(8):
                s1_compute(b, k)
            for m in range(4):
                s2_compute(b, m)
    nc.compile()
    return nc


def _get_program():
    if "nc" not in _CACHE:
        _CACHE["nc"] = _build_program()
    return _CACHE["nc"]


def _ensure_ntff_hook():
    """Bridge the axon NTFF profile hook when the image's antenv lacks it."""
    import sys, types

    try:
        from antenv.axon_hooks import get_axon_ntff_profile_hook  # noqa: F401

        return
    except ImportError:
        pass
    try:
        from trn_agent_boot.trn_boot import _ntff_profile_via_ctypes

        hook = _ntff_profile_via_ctypes("/opt/axon/libaxon_pjrt.so")
        mod = types.ModuleType("antenv.axon_hooks")
        _h = {"hook": hook}
        mod.set_axon_ntff_profile_hook = lambda h: _h.__setitem__("hook", h)
        mod.get_axon_ntff_profile_hook = lambda: _h["hook"]
        sys.modules["antenv.axon_hooks"] = mod
        import antenv

        antenv.axon_hooks = mod
    except Exception:
        pass


def kernel(x, factorL, factorR, bias):
    global LAST_RESULT
    import ml_dtypes
    from concourse.bass_utils import run_bass_kernel_spmd

    BF = ml_dtypes.bfloat16
    x = np.asarray(x, dtype=np.float32)
    factorL = np.asarray(factorL, dtype=np.float32)
    factorR = np.asarray(factorR, dtype=np.float32)
    bias = np.asarray(bias, dtype=np.float32)

    # host-side marshalling (not device-timed)
    xt = np.ascontiguousarray(x.reshape(TOK, 4096).T.astype(BF))  # (4096, 8192)

    # stage-1 output channel permute, k-dependent:
    #   q'' = 128*qc + 64*hh + rr  ->  l = 2*qc + ((hh + k) % 2), q = 8*rr + l
    qpp = np.arange(512)
    qcv, hh, rr = qpp // 128, (qpp % 128) // 64, qpp % 64
    w1T = factorL.transpose(0, 2, 1)  # (k, p, q)
    w1p = np.empty((8, 512, 512), np.float32)  # (k, p, q'')
    for k in range(8):
        l = 2 * qcv + ((hh + k) % 2)
        w1p[k] = w1T[k][:, 8 * rr + l]
    w1dev = np.ascontiguousarray(
        w1p.reshape(8, 4, 128, 4, 128)
        .transpose(2, 0, 1, 3, 4)
        .reshape(128, 16384)
        .astype(BF)
    )

    # stage-2 weights: tile m holds l = 2m (part-half (k%2)) and l = 2m+1
    # (part-half (k+1)%2); chunk (l, k) = rows r in [64k, 64k+64)
    w2T = factorR.transpose(0, 2, 1)  # (l, r, s)
    w2dev = np.zeros((128, 16384), np.float32)
    for l in range(8):
        m = l // 2
        for k in range(8):
            h2 = 64 * ((l + k) % 2)
            w2dev[h2 : h2 + 64, m * 4096 + k * 512 : m * 4096 + (k + 1) * 512] = (
                w2T[l, 64 * k : 64 * k + 64, :]
            )
    w2dev = np.ascontiguousarray(w2dev.astype(BF))

    biasdev = np.ascontiguousarray(
        bias.reshape(4, 128, 8).transpose(1, 2, 0).reshape(128, 32)
    )

    in_maps = [
        {
            "x": np.ascontiguousarray(xt[:, c * TPC : (c + 1) * TPC]),
            "w1": w1dev,
            "w2": w2dev,
            "bias": biasdev,
        }
        for c in range(NCORES)
    ]
    nc = _get_program()
    trace = os.environ.get("BUTTERFLY_TRACE", "0") == "1"
    if trace:
        _ensure_ntff_hook()
    LAST_RESULT = run_bass_kernel_spmd(
        nc, in_maps, list(range(NCORES)), trace=trace
    )
    yt = np.concatenate(
        [LAST_RESULT.results[c]["out"] for c in range(NCORES)], axis=1
    )  # (4096, 8192) bf16
    return np.ascontiguousarray(yt.T).astype(np.float32).reshape(4, 2048, 4096)
